# revision 60
# baseline (speedup 1.0000x reference)
"""Trainium2 Bass kernel for nn_MultiHeadAttention_20066087207431.

Reference computation (B=2, S=2048, NV=1024, H=16, DH=64):
    Q = Sq @ Wq_w.T + Wq_b ; K = Sk @ Wq_w.T + Wq_b ; V = Sv @ Wq_w.T + Wq_b
    per (batch, head):  qk = Q K^T / sqrt(DH) ;  Y = qk @ V
    X = softmax(Y, axis=-1)          # softmax AFTER the second matmul (quirk)
    out = X @ out_w.T + out_b

Key algebraic optimizations:
  - No softmax between the two attention matmuls, so (Q K^T) V == Q (K^T V).
    K^T V is only (64, 64) per head, which collapses ~34 GFLOP of score math
    into ~1 GFLOP and removes the (S, S) score matrices entirely.
  - Softmax rows sum to exactly 1 per head (16 over the 1024 columns), so
    the output bias folds into the output weights on the host:
    X @ (out_w + out_b/16 * ones).T == X @ out_w.T + out_b.  The final PSUM
    evacuation is then a plain copy on the ACT engine.

Sharding: 8 cores; core c handles batch b = c // 4 and a 512-token slice of S
(data-parallel over batch*sequence).  Each core computes local partial
M = K_loc^T V_loc / sqrt(DH) for all 16 heads; a 256 KB AllReduce within each
4-core batch group completes the sum over S.  Everything else (projections,
Y = Q M, softmax, output projection) is local to the core.  Weights are
replicated.

Dtype: fp16 everywhere on the PE.  fp16 has the same 10-bit mantissa as TF32
(f32r), so accuracy matches the TF32 baseline, while:
  - HBM traffic halves (inputs/weights/output are 2 bytes),
  - matmuls run at 1 cycle/row with NO N>=256 restriction (f32r needs
    N>=256 else 4x penalty), so the N=128 attention matmuls are full rate,
  - PE transposes run at 1.0 cycles/row instead of 1.5.
PSUM accumulation stays fp32; biases are added in fp32 during PSUM
evacuation, then rounded once to fp16.

Schedule: K proj -> V proj/M interleaved per column half (each M half's
AllReduce launches ~10 us apart so the DRAM-staged hops hide under later
PE work) -> token-major Q proj with all four Y = Q M chunks and their
softmaxes woven in -> a trailing transpose/output-projection stream
T0 T1 O0 T2 O1 T3 O2 O3 whose inputs are all ready as the PE arrives,
keeping the PE gapless (gaps also drop it to the 1.2 GHz p-state for
3 us).  The Q bias rides into Y as a rank-1 PE matmul (ones (x) beta,
beta = bq^T blockdiag(M)), so Q's PSUM evacuation is a plain ACT copy
and the DVE runs only the softmax chains in the tail.  Softmax is split
into head-halves: max/sub for both halves first (DVE, overlapping ACT's
exps), then sum/recip/normalize per half, with one normalize-mult per
chunk on the otherwise-idle GPSIMD.
"""

import os
import sys

import numpy as np

for _p in ("/opt/trn_rl_repo", "/root/.axon_site/_ro/trn_rl_repo"):
    if os.path.isdir(_p) and _p not in sys.path:
        sys.path.insert(0, _p)

import concourse.bass as bass  # noqa: E402
import concourse.mybir as mybir  # noqa: E402
import concourse.tile as tile  # noqa: E402
from concourse import bacc  # noqa: E402
from concourse.bass_utils import run_bass_kernel_spmd  # noqa: E402
from concourse.masks import make_identity  # noqa: E402

F32 = mybir.dt.float32
F16 = mybir.dt.float16
AF = mybir.ActivationFunctionType
ALU = mybir.AluOpType

B, S, NV = 2, 2048, 1024
H, DH = 16, 64
P = 128
NCORES = 8
S_LOC = S // 4          # tokens per core: 512
KC = NV // P            # contraction chunks: 8
MC = S_LOC // P         # token chunks per core: 4
FC = NV // P            # output-feature chunks: 8
SCALE = 1.0 / 8.0       # 1 / sqrt(DH)

REPLICA_GROUPS = [[0, 1, 2, 3], [4, 5, 6, 7]]


def build_nc(collective=True):
    nc = bacc.Bacc("TRN2", target_bir_lowering=False, debug=False,
                   num_devices=NCORES if collective else 1)

    sqT = nc.dram_tensor("sqT", [NV, S_LOC], F16, kind="ExternalInput").ap()
    skT = nc.dram_tensor("skT", [NV, S_LOC], F16, kind="ExternalInput").ap()
    svT = nc.dram_tensor("svT", [NV, S_LOC], F16, kind="ExternalInput").ap()
    wqT = nc.dram_tensor("wqT", [NV, NV], F16, kind="ExternalInput").ap()
    owT = nc.dram_tensor("owT", [NV, NV], F16, kind="ExternalInput").ap()
    bq = nc.dram_tensor("bq", [NV], F32, kind="ExternalInput").ap()
    z = nc.dram_tensor("z", [S_LOC, NV], F16, kind="ExternalOutput").ap()

    with tile.TileContext(nc) as tc:
        _emit(nc, tc, sqT, skT, svT, wqT, owT, bq, z,
              collective=collective)

    nc.compile()
    return nc


def _emit(nc, tc, sqT, skT, svT, wqT, owT, bq, z, collective=True):
    from contextlib import ExitStack

    with ExitStack() as ctx:
        persist = ctx.enter_context(tc.tile_pool(name="persist", bufs=1))
        scratch = ctx.enter_context(tc.tile_pool(name="scratch", bufs=2))
        psproj = ctx.enter_context(
            tc.tile_pool(name="psproj", bufs=2, space="PSUM"))
        dram = ctx.enter_context(tc.tile_pool(name="dram", bufs=1, space="DRAM"))

        # ---- constant / weight loads ------------------------------------
        # DMA issue order is the critical path at kernel start.  Chunks keep
        # >=512-byte DRAM lines (256-byte-line DMAs run at half bandwidth).
        # The first K-projection output is emitted as two N=256 column
        # groups so the PE can start after sk half 0 + Wq cols 0:256
        # (~4.9 us).  Everything else queues in consumption order; out_w
        # last.
        sk_sb = persist.tile([P, KC, S_LOC], F16)
        sk_view = skT.rearrange("(kc p) m -> p kc m", p=P)
        nc.sync.dma_start(sk_sb[:, 0:KC // 2], sk_view[:, 0:KC // 2])
        wq_sb = persist.tile([P, KC, NV], F16)   # wq_sb[p,kc,n] = Wq_w[n, kc*128+p]
        wq_view = wqT.rearrange("(kc p) n -> p kc n", p=P)
        nc.sync.dma_start(wq_sb[:, 0:KC // 2, 0:512],
                          wq_view[:, 0:KC // 2, 0:512])
        nc.sync.dma_start(sk_sb[:, KC // 2:], sk_view[:, KC // 2:])
        nc.sync.dma_start(wq_sb[:, KC // 2:, 0:512],
                          wq_view[:, KC // 2:, 0:512])

        bqp_sb = persist.tile([P, FC], F32)      # per-partition view for QT bias
        nc.sync.dma_start(bqp_sb[:], bq.rearrange("(c p) -> p c", p=P))
        bqr_sb = persist.tile([1, NV], F32)      # bias as a single row
        nc.sync.dma_start(bqr_sb[:], bq[None, :])
        # bias replicated across partitions, for the free-dim bias add on K/V
        bqb_sb = persist.tile([P, NV], F32)
        nc.gpsimd.partition_broadcast(bqb_sb[:], bqr_sb[:])

        ident_sb = persist.tile([P, P], F16)
        make_identity(nc, ident_sb[:])

        # scratch operand for PE warm-up matmuls (see emit_warm below)
        dummy_sb = persist.tile([P, 512], F16)
        nc.vector.memset(dummy_sb[:], 0.0)

        nc.sync.dma_start(wq_sb[:, :, 512:NV], wq_view[:, :, 512:NV])
        sv_sb = persist.tile([P, KC, S_LOC], F16)
        sv_view = svT.rearrange("(kc p) m -> p kc m", p=P)
        nc.sync.dma_start(sv_sb[:, 0:KC // 2], sv_view[:, 0:KC // 2])
        nc.sync.dma_start(sv_sb[:, KC // 2:], sv_view[:, KC // 2:])
        sq_sb = persist.tile([P, KC, S_LOC], F16)
        nc.sync.dma_start(sq_sb[:], sqT.rearrange("(kc p) m -> p kc m", p=P))
        ow_sb = persist.tile([P, KC, NV], F16)   # out_w + ob/16 (host-folded)
        ow_view = owT.rearrange("(kc p) n -> p kc n", p=P)
        nc.sync.dma_start(ow_sb[:, 0:KC // 2], ow_view[:, 0:KC // 2])
        nc.sync.dma_start(ow_sb[:, KC // 2:], ow_view[:, KC // 2:])

        q_sb = persist.tile([P, FC, S_LOC], F16)  # Q^T: feature on partition
        k_sb = persist.tile([P, MC, NV], F16)     # K natural: token on partition
        v_sb = persist.tile([P, MC, NV], F16)
        m_sb = persist.tile([P, NV], F16)         # local K^T V / 8, block-diag
        mr_sb = persist.tile([P, NV], F16)        # after AllReduce

        # ---- K, V projections in natural layout (token on partition) ----
        #   K[m, n] = sum_k Sk[m, k] Wq[n, k] + bq[n]
        #   lhsT = SkT chunk (k on part, token free) ; rhs = WqT chunk
        #   bias (varies along the free dim) folds into the PSUM evacuation
        #   (DVE: GPSIMD cannot read PSUM, ACT cannot add a free-dim bias)
        # half-outer iteration: all of Wq's first column half is consumed
        # before the second half's DMA needs to have landed
        def emit_proj(src, dst, half, first=False):
            nsl = slice(half * 512, (half + 1) * 512)
            for mc in range(MC):
                ps = psproj.tile([P, 512], F32, tag="proj")
                # first output in N=256 column groups: the second group
                # waits on Wq cols 256:512 while the first streams
                nq = 2 if (first and mc == 0) else 1
                for q in range(nq):
                    w = 512 // nq
                    psl = slice(q * w, (q + 1) * w)
                    qsl = slice(half * 512 + q * w, half * 512 + (q + 1) * w)
                    for kc in range(KC):
                        nc.tensor.matmul(
                            ps[:, psl],
                            (src[:, kc, mc * P:(mc + 1) * P]),
                            (wq_sb[:, kc, qsl]),
                            start=(kc == 0), stop=(kc == KC - 1))
                nc.vector.tensor_tensor(
                    dst[:, mc, nsl], ps[:], bqb_sb[:, nsl], ALU.add)

        # local M = K^T V / 8 for one head-half (feature chunks hh*4..hh*4+3;
        # chunk fc holds heads 2fc, 2fc+1 in its 64x64 diagonal blocks; fp16
        # runs full rate at N=128 so only the fc x fc product is computed),
        # then AllReduce that half within the batch group (128 KB fp16).
        # Splitting by half starts the first collective ~10 us earlier, so
        # its three serial DMA hops (~2 us DGE latency each -- DRAM staging
        # is mandatory, SBUF collectives are broken) hide under the
        # Q projection and never gate Y.
        def emit_m(psm, hh, m_in, m_out):
            for f in range(4):
                fc = hh * 4 + f
                ps = psm.tile([P, P], F32, tag="m")
                for mc in range(MC):
                    nc.tensor.matmul(
                        ps[:],
                        (k_sb[:, mc, fc * P:(fc + 1) * P]),
                        (v_sb[:, mc, fc * P:(fc + 1) * P]),
                        start=(mc == 0), stop=(mc == MC - 1))
                # DVE, not ACT: M gates the collective
                nc.vector.tensor_scalar_mul(
                    m_sb[0:64, fc * P:fc * P + 64],
                    ps[0:64, 0:64], SCALE)
                nc.vector.tensor_scalar_mul(
                    m_sb[64:128, fc * P + 64:fc * P + 128],
                    ps[64:128, 64:128], SCALE)
            hsl = slice(hh * 512, (hh + 1) * 512)
            nc.sync.dma_start(m_in[:], m_sb[:, hsl])
            if collective:
                nc.gpsimd.collective_compute(
                    "AllReduce", ALU.add,
                    replica_groups=REPLICA_GROUPS,
                    ins=[m_in.opt()], outs=[m_out.opt()])
            else:  # single-core perf-model variant: same traffic
                nc.sync.dma_start(m_out[:], m_in[:])
            nc.sync.dma_start(mr_sb[:, hsl], m_out[:])

        # PE warm-up: the first ~4.9 us are DMA-bound (sk half 0 + a quarter
        # of Wq must land before the first real matmul) and the cost model
        # keeps the PE at its 1.2 GHz p-state until it has run 3 us
        # continuously.  Dummy matmuls on scratch data fill the idle start
        # window so the K projection begins already ramped to 2.4 GHz.
        with tc.tile_pool(name="psw", bufs=1, space="PSUM") as psw:
            wps = psw.tile([P, 512], F32, tag="warm", bufs=1)
            for _ in range(10):
                nc.tensor.matmul(wps[:], dummy_sb[:, 0:P], dummy_sb[:],
                                 start=True, stop=True)

        # K projection, column half 0, in two kc-phases: kc 0..3 for all
        # token chunks (needs only sk half 0 + the first quarter of Wq,
        # ~1 MB of feed), then kc 4..7 accumulating into the same PSUM
        # tiles.  This keeps the PE on useful work through the DMA-bound
        # start instead of waiting for 1.5 MB before the first output.
        with tc.tile_pool(name="pska", bufs=1, space="PSUM") as pska:
            ka = [pska.tile([P, 512], F32, name=f"ka{mc}", bufs=1)
                  for mc in range(MC)]
            for kh in range(2):
                for mc in range(MC):
                    for kc in range(kh * 4, kh * 4 + 4):
                        nc.tensor.matmul(
                            ka[mc][:],
                            (sk_sb[:, kc, mc * P:(mc + 1) * P]),
                            (wq_sb[:, kc, 0:512]),
                            start=(kc == 0), stop=(kc == KC - 1))
                    if kh == 1:
                        nc.vector.tensor_tensor(
                            k_sb[:, mc, 0:512], ka[mc][:], bqb_sb[:, 0:512],
                            ALU.add)
        emit_proj(sk_sb, k_sb, 1)
        nc.vector.memset(m_sb[:], 0.0)
        m_io = [(dram.tile([P, 512], F16, name=f"m_in{h}"),
                 dram.tile([P, 512], F16, name=f"m_out{h}"))
                for h in range(2)]
        with tc.tile_pool(name="psm", bufs=2, space="PSUM") as psm:
            emit_proj(sv_sb, v_sb, 0)
            emit_m(psm, 0, *m_io[0])
            emit_proj(sv_sb, v_sb, 1)
            emit_m(psm, 1, *m_io[1])

        # ---- Q projection (token-major) + attention tail, interleaved ---
        # Q is projected one 128-token chunk at a time so that Y(mc) and its
        # softmax can start while later Q chunks still stream on the PE: the
        # softmax latency hides entirely under Q-projection matmuls instead
        # of stalling the PE at the handoff.  PE emission order:
        #   Q0 Q1 Y0 Q2 Y1 Q3 Y2 T0 O0 Y3 T1 O1 T2 O2 T3 O3
        # Softmax per head-half: max/sub/sum/recip on DVE, exp on ACT,
        # normalize-mult half 0 on GPSIMD, half 1 on DVE.  Y PSUM is one
        # bank per (mc, half) with tags reused mod 2.
        x_sb = persist.tile([P, MC, NV], F16)       # softmax output, natural
        xT_sb = persist.tile([P, KC, S_LOC], F16)   # X^T: feature on partition
        z_sb = persist.tile([P, MC, NV], F16)       # output, natural layout
        zv = z.rearrange("(mc p) n -> p mc n", p=P)
        bqc16_sb = persist.tile([P, FC], F16)       # bq as fp16 columns
        nc.vector.tensor_copy(bqc16_sb[:], bqp_sb[:])
        ones_sb = persist.tile([1, P], F16)
        nc.vector.memset(ones_sb[:], 1.0)
        beta_sb = persist.tile([1, NV], F16)        # bq^T blockdiag(M)

        HH = H // 2
        with tc.tile_pool(name="psy", bufs=2, space="PSUM") as psy:

            def emit_q(tc_):
                # one 128-token chunk of Q^T (no bias) for all 8 feature
                # chunks, as two [P, 4, 128] PSUM groups sharing the "proj"
                # tag.  The Q bias is folded into Y as a rank-1 update
                # (see emit_beta), so the evacuation is a plain ACT copy
                # and the DVE stays free for the softmax chains.
                for half in range(2):
                    ps = psproj.tile([P, 512], F32, tag="proj")
                    pv = ps.rearrange("p (f m) -> p f m", m=P)
                    for f in range(4):
                        fc = half * 4 + f
                        for kc in range(KC):
                            nc.tensor.matmul(
                                pv[:, f],
                                (wq_sb[:, kc, fc * P:(fc + 1) * P]),
                                (sq_sb[:, kc, tc_ * P:(tc_ + 1) * P]),
                                start=(kc == 0), stop=(kc == KC - 1))
                    nc.scalar.copy(
                        q_sb[:, half * 4:(half + 1) * 4,
                             tc_ * P:(tc_ + 1) * P], pv)

            def emit_beta(hh):
                # beta[(h,d2)] = sum_d1 bq[(h,d1)] M_h[d1,d2]: with the
                # block-diagonal mr layout this is one [1 x 128] matmul per
                # feature chunk (lhsT = the bias column, rhs = the M block).
                # Y then gets bias' = ones (x) beta as a rank-1 accumulate:
                # softmax(Q M) with Q = Q0 + ones (x) bq equals
                # softmax(Q0 M + ones (x) beta).
                bps = psproj.tile([P, 512], F32, tag="proj")
                for f in range(4):
                    fc = hh * 4 + f
                    nc.tensor.matmul(
                        bps[0:1, f * P:(f + 1) * P],
                        bqc16_sb[:, fc:fc + 1],
                        mr_sb[:, fc * P:(fc + 1) * P],
                        start=True, stop=True)
                nc.scalar.copy(beta_sb[:, hh * 512:(hh + 1) * 512], bps[0:1, :])

            def emit_y(mc):
                # one PSUM bank per (mc, head-half), tags reused mod 2: Y(mc)
                # never serializes against softmax(mc-1), and Y(mc+2)'s WAR
                # on softmax(mc)'s reads resolves long before it's emitted
                ys = []
                for hh in range(2):
                    yps = psy.tile([P, 512], F32, tag=f"y{mc % 2}{hh}",
                                   bufs=1)
                    for f in range(4):
                        fc = hh * 4 + f
                        nc.tensor.matmul(
                            yps[:, f * P:(f + 1) * P],
                            q_sb[:, fc, mc * P:(mc + 1) * P],
                            mr_sb[:, fc * P:(fc + 1) * P],
                            start=True, stop=False)
                        # rank-1 Q-bias contribution (see emit_beta)
                        nc.tensor.matmul(
                            yps[:, f * P:(f + 1) * P],
                            ones_sb[:],
                            beta_sb[:, fc * P:(fc + 1) * P],
                            start=False, stop=True)
                    ys.append(yps)
                return ys

            def emit_maxsub(mc, ys):
                # phase 1 of softmax(mc): negated max + subtract for both
                # head-halves (DVE), exp on ACT.  The max MUST be per
                # (token, head): Y's scale varies per head (~ ||M_h||), so
                # a shared per-token max makes weak heads underflow to an
                # all-zero row (0/0 after normalization).
                es = []
                for hh in range(2):
                    y3 = ys[hh].rearrange("p (h d) -> p h d", d=DH)
                    nmx = scratch.tile([P, HH], F32, tag=f"nmx{hh}", bufs=4)
                    nc.vector.reduce_max(nmx[:], y3,
                                         axis=mybir.AxisListType.X,
                                         negate=True)
                    e_sb = scratch.tile([P, 512], F16, tag=f"e{hh}", bufs=4)
                    e3 = e_sb.rearrange("p (h d) -> p h d", d=DH)
                    nc.vector.tensor_tensor(
                        e3, y3,
                        nmx[:, :, None].to_broadcast((P, HH, DH)), ALU.add)
                    nc.scalar.activation(e_sb[:], e_sb[:], AF.Exp)
                    es.append(e3)
                return es

            def emit_srm(mc, es):
                # phase 2: per half, sum + reciprocal + normalize-mult.
                # Mult engine choice: chunks 1 and 2 put BOTH mults on
                # GPSIMD -- their transposes run late in the PE stream so
                # the slow GPSIMD mult is covered, and more importantly the
                # last DVE op of these chains becomes a cheap reciprocal,
                # so the framework's DVE-watermark dependency on the next
                # Y chunk fires ~1.3 us earlier.  Chunk 0 keeps half 1 on
                # the DVE (its x gates T0, the first transpose); the last
                # chunk keeps both on the DVE (no later PE work hides
                # GPSIMD latency).
                x3 = x_sb[:, mc, :].rearrange("p (h d) -> p h d", d=DH)
                for hh in range(2):
                    hsl = slice(hh * HH, (hh + 1) * HH)
                    e3 = es[hh]
                    sm = scratch.tile([P, HH], F32, tag=f"sm{hh}")
                    nc.vector.reduce_sum(sm[:], e3, axis=mybir.AxisListType.X)
                    rc = scratch.tile([P, HH], F32, tag=f"rc{hh}")
                    nc.vector.reciprocal(rc[:], sm[:])
                    rcb = rc[:, :, None].to_broadcast((P, HH, DH))
                    if hh == 0 and mc < MC - 1:
                        nc.gpsimd.tensor_tensor(x3[:, hsl], e3, rcb, ALU.mult)
                    else:
                        nc.vector.tensor_tensor(x3[:, hsl], e3, rcb, ALU.mult)

            def emit_t(mc):
                # transpose token chunk mc back to feature-on-partition.
                # The two evacuation copies run in parallel on DVE and ACT
                # so the chunk's X^T is ready ~0.8 us after the transposes.
                pst = psy.tile([P, 2, 512], F16, tag="tp", bufs=2)
                for th in range(2):
                    for f in range(4):
                        fc = th * 4 + f
                        nc.tensor.transpose(
                            pst[:, th, f * P:(f + 1) * P],
                            x_sb[:, mc, fc * P:(fc + 1) * P], ident_sb[:])
                    nc.scalar.copy(
                        xT_sb[:, th * 4:(th + 1) * 4, mc * P:(mc + 1) * P],
                        pst[:, th].rearrange("p (fc m) -> p fc m", m=P))

            def emit_o(mc):
                #   z[m, n] = sum_k X[m, k] (out_w[n, k] + ob[n]/16)
                #   lhsT = X^T chunk (feat on part, token free); rhs = owT
                for half in range(2):
                    nsl = slice(half * 512, (half + 1) * 512)
                    ps = psproj.tile([P, 512], F32, tag="proj")
                    for kc in range(KC):
                        nc.tensor.matmul(
                            ps[:],
                            (xT_sb[:, kc, mc * P:(mc + 1) * P]),
                            (ow_sb[:, kc, nsl]),
                            start=(kc == 0), stop=(kc == KC - 1))
                    # the DVE is idle by the last chunk, so its first-half
                    # copy runs there, in parallel with ACT's second half
                    if mc == MC - 1 and half == 0:
                        nc.vector.tensor_copy(z_sb[:, mc, nsl], ps[:])
                    else:
                        nc.scalar.copy(z_sb[:, mc, nsl], ps[:])
                    nc.sync.dma_start(zv[:, mc, nsl], z_sb[:, mc, nsl])

            # All Y chunks run early (their PSUM banks rotate mod 2, freed
            # by each chunk's subs); the trailing PE stream T0 O0 .. T3 O3
            # then has every input ready as it arrives, and the last
            # chunk's serial chain (sm3 -> T3 -> O3 -> evac) starts ~13 us
            # before the PE gets there.
            emit_q(0)
            emit_q(1)
            emit_q(2)
            # beta 1 needs mr half 1, ready only ~here; beta borrows a
            # "proj" PSUM buffer (it only writes partition 0's row)
            emit_beta(0)
            emit_beta(1)
            ms0 = emit_maxsub(0, emit_y(0))
            ms1 = emit_maxsub(1, emit_y(1))
            emit_srm(0, ms0)
            emit_q(3)
            emit_srm(1, ms1)
            ms2 = emit_maxsub(2, emit_y(2))
            emit_srm(2, ms2)
            ms3 = emit_maxsub(3, emit_y(3))
            emit_srm(3, ms3)
            emit_t(0)
            emit_t(1)
            emit_o(0)
            emit_t(2)
            emit_o(1)
            emit_t(3)
            emit_o(2)
            emit_o(3)


_NC_CACHE = None


def _get_nc():
    global _NC_CACHE
    if _NC_CACHE is None:
        _NC_CACHE = build_nc()
    return _NC_CACHE


def make_in_maps(Sq, Sk, Sv, Wq_w, Wq_b, out_w, out_b):
    wqT = np.ascontiguousarray(
        np.asarray(Wq_w, dtype=np.float32).T.astype(np.float16))
    # output bias folded into the output weights: softmax rows sum to
    # exactly 1 per head and there are 16 heads, so X @ (ow + ob/16).T
    # == X @ ow.T + ob
    ow_f = np.asarray(out_w, dtype=np.float32) + \
        np.asarray(out_b, dtype=np.float32)[:, None] / np.float32(H)
    owT = np.ascontiguousarray(ow_f.T.astype(np.float16))
    bq = np.asarray(Wq_b, dtype=np.float32)
    in_maps = []
    for c in range(NCORES):
        b, q = c // 4, c % 4
        rows = slice(q * S_LOC, (q + 1) * S_LOC)
        in_maps.append({
            "sqT": np.ascontiguousarray(
                np.asarray(Sq[b, rows], np.float32).T.astype(np.float16)),
            "skT": np.ascontiguousarray(
                np.asarray(Sk[b, rows], np.float32).T.astype(np.float16)),
            "svT": np.ascontiguousarray(
                np.asarray(Sv[b, rows], np.float32).T.astype(np.float16)),
            "wqT": wqT, "owT": owT, "bq": bq,
        })
    return in_maps


def gather_output(results):
    out = np.empty((B, S, NV), dtype=np.float32)
    for c in range(NCORES):
        b, q = c // 4, c % 4
        out[b, q * S_LOC:(q + 1) * S_LOC, :] = results[c]["z"].astype(
            np.float32)
    return out


def kernel(Sq, Sk, Sv, Wq_w, Wq_b, out_w, out_b, **_unused):
    nc = _get_nc()
    in_maps = make_in_maps(Sq, Sk, Sv, Wq_w, Wq_b, out_w, out_b)
    res = run_bass_kernel_spmd(nc, in_maps, core_ids=list(range(NCORES)))
    return gather_output(res.results)
